# revision 1
# baseline (speedup 1.0000x reference)
"""DMSA (dual-modal channel cross-attention) Trainium2 kernel — v2.

Sharding: 8 cores = 2 batches x 4 bands of 32 image rows. Each core
computes its band fully; the channel attention's per-head Gram matrices
(contraction over all n = h*w tokens, with l2-normalization folded in
via the Gram diagonal) are summed with one AllReduce per 4-core group.

Device layout: channel-major activations [128 partitions, 2 channel
halves, tokens]. Stage-1 runs on an unpadded 36x128 ext-row grid
(9 tiles x 512 tokens); v is spilled to a width-padded 36x130 DRAM
grid so both 3x3 depthwise convs read taps as plain offset views.

Engines: big matmuls f32r (~1e-4); q/k hidden + Gram in bf16 (feeds
only softmax(cos-sim) logits); conv1 on DVE (overlaps the AllReduce);
conv2 folded into the output projection's PSUM accumulation as 9
diagonal-matrix matmuls on PE.
"""
import numpy as np
import ml_dtypes
from contextlib import ExitStack

import concourse.bass as bass
import concourse.tile as tile
import concourse.mybir as mybir
from concourse import bacc
from concourse.bass_utils import run_bass_kernel_spmd

F32 = mybir.dt.float32
F32R = mybir.dt.float32r
BF16 = mybir.dt.bfloat16
AF = mybir.ActivationFunctionType
OP = mybir.AluOpType

B, H, W, C = 2, 128, 128, 256
HEADS, DH = 8, 32
RB = 32             # image rows per core
ER = RB + 4         # ext rows
WP = W + 2          # padded width (conv grid)
GN = ER * WP        # padded tokens (v spill grid) = 4680
EN = ER * W         # unpadded ext tokens (stage-1 grid) = 4608
NV = RB * W         # valid tokens = 4096
NT = 9              # stage-1 tiles (4 ext rows each)
LRELU_A = 0.01
# conv1 chunk g-row ranges and the stage-1 tile after which each may run
C1CHUNKS = [(0, 6, 1), (6, 12, 3), (12, 18, 4), (18, 24, 6), (24, 30, 7),
            (30, 34, None)]  # None -> after the collective

_CACHED = {}


def _nc_build():
    nc = bacc.Bacc(num_devices=8)

    din = {}
    def inp(name, shape, dt=F32R):
        din[name] = nc.dram_tensor(name, list(shape), dt, kind="ExternalInput")
        return din[name]

    xin = inp("xin", [128, 2, EN])
    yin = inp("yin", [128, 2, EN])
    inp("fxw1T", [128, 4, 2, 128])
    inp("fyw1T", [128, 4, 2, 128])
    inp("qw1T", [128, 2, 2, 128])
    inp("kxw1T", [128, 2, 2, 128])
    inp("kyw1T", [128, 2, 2, 128])
    inp("vw1T", [128, 2, 2, 128])
    inp("vw2T", [128, 2, 2, 128])
    inp("qw2T", [128, 2, 256], BF16)
    inp("kw2T", [128, 2, 256], BF16)
    inp("pxwT", [128, 2, 256])
    inp("pywT", [128, 2, 256])
    inp("dw2", [128, 2, 9, 128], BF16)      # conv2 taps as diagonal lhsT
    inp("blk128", [128, 128])               # kron(eye(4), ones(32,32))
    inp("eye32r", [128, 32], F32)           # tile(eye(32), (4,1))
    for nm in ("bfx", "bfy", "bq", "bkx", "bky", "bv", "obx", "oby", "b1c",
               "rx_exp", "ry_exp"):
        inp(nm, [128, 2], F32)
    inp("w1c", [128, 2, 9], F32)            # conv1 taps (DVE)
    inp("gm0", [128, 1], F32)
    inp("gm33", [128, 1], F32)

    out_x = nc.dram_tensor("out_x", [128, 2, NV], F32, kind="ExternalOutput")
    out_y = nc.dram_tensor("out_y", [128, 2, NV], F32, kind="ExternalOutput")
    vsp_x = nc.dram_tensor("vsp_x", [128, 2, GN], F32R, kind="Internal")
    vsp_y = nc.dram_tensor("vsp_y", [128, 2, GN], F32R, kind="Internal")
    cc_in = nc.dram_tensor("cc_in", [HEADS, 128, 128], F32, kind="Internal")
    cc_out = nc.dram_tensor("cc_out", [HEADS, 128, 128], F32, kind="Internal")

    with tile.TileContext(nc) as tc, ExitStack() as ctx:
        wp = ctx.enter_context(tc.tile_pool(name="wp", bufs=1))
        io = ctx.enter_context(tc.tile_pool(name="io", bufs=2))
        hidF = ctx.enter_context(tc.tile_pool(name="hidF", bufs=2))
        hidQ = ctx.enter_context(tc.tile_pool(name="hidQ", bufs=2))
        hidV = ctx.enter_context(tc.tile_pool(name="hidV", bufs=2))
        stk = ctx.enter_context(tc.tile_pool(name="stk", bufs=2))
        sm = ctx.enter_context(tc.tile_pool(name="sm", bufs=1))
        gb = ctx.enter_context(tc.tile_pool(name="gb", bufs=1))
        cvp = ctx.enter_context(tc.tile_pool(name="cvp", bufs=2))
        ot = ctx.enter_context(tc.tile_pool(name="ot", bufs=2))
        psA = ctx.enter_context(tc.tile_pool(name="psA", bufs=2, space="PSUM"))
        psQ = ctx.enter_context(tc.tile_pool(name="psQ", bufs=2, space="PSUM"))
        psG = ctx.enter_context(tc.tile_pool(name="psG", bufs=1, space="PSUM"))

        w = {}
        for name, h in din.items():
            if name in ("xin", "yin"):
                continue
            t = wp.tile(list(h.shape), h.dtype, tag=f"w_{name}")
            nc.sync.dma_start(t[:], h.ap())
            w[name] = t

        # one-time zeroing of the v-spill pad columns
        zt = wp.tile([128, 2, ER], F32R, tag="zt")
        nc.vector.tensor_scalar_mul(zt.bitcast(F32)[:], zt.bitcast(F32)[:],
                                    0.0)
        for vsp in (vsp_x, vsp_y):
            vv = vsp.ap().rearrange("p a (r c) -> p a r c", c=WP)
            nc.sync.dma_start(vv[:, :, :, 0], zt[:])
            nc.sync.dma_start(vv[:, :, :, WP - 1], zt[:])

        gram0 = psG.tile([128, 512], F32, tag="gram0")
        gram1 = psG.tile([128, 512], F32, tag="gram1")
        grams = [gram0, gram1]

        gx = gb.tile([128, 2, ER - 2, WP], BF16, tag="gx")
        gy = gb.tile([128, 2, ER - 2, WP], BF16, tag="gy")
        nc.scalar.memzero(gx[:])
        nc.scalar.memzero(gy[:])
        TAPS = [(dr, dc) for dr in (-1, 0, 1) for dc in (-1, 0, 1)]

        def conv1_chunk(gbuf, vsp, g0, g1):
            """DVE 9-tap conv1 for g rows [g0, g1) + Gelu evict into gbuf."""
            vr0, vr1 = g0, min(g1 + 2, ER)
            nr = g1 - g0
            vc = cvp.tile([128, 2, 8, WP], F32R, tag="vc")
            nc.sync.dma_start(vc[:, :, :vr1 - vr0, :],
                              vsp.ap()[:, :, vr0 * WP:vr1 * WP])
            for g in range(2):
                acc = cvp.tile([128, 6, 128], F32, tag="cacc")
                for i, (dr, dc) in enumerate(TAPS):
                    src = vc[:, g, g0 + 1 + dr - vr0:g0 + 1 + dr - vr0 + nr,
                             1 + dc:129 + dc]
                    if i == 0:
                        nc.vector.tensor_scalar_mul(acc[:, :nr, :], src,
                                                    w["w1c"][:, g, 0:1])
                    else:
                        nc.vector.scalar_tensor_tensor(
                            acc[:, :nr, :], src, w["w1c"][:, g, i:i + 1],
                            acc[:, :nr, :], OP.mult, OP.add)
                nc.scalar.activation(gbuf[:, g, g0:g1, 1:129], acc[:, :nr, :],
                                     AF.Gelu, bias=w["b1c"][:, g:g + 1])

        # ================= stage 1 =================
        vrow = 0

        def mlp1(srcs, w1T, nk, bias, tag, pool, dt, lo=0, n=512):
            """hidden = lrelu(srcs @ w1T + b); paired-bank PSUM."""
            ht = pool.tile([128, 2, 512], dt, tag=tag)
            ps = psA.tile([128, 2, 512], F32, tag="psA")
            for mh in range(2):
                for k in range(nk):
                    src = srcs[k // 2][:, k % 2, lo:lo + n] if len(srcs) > 1 \
                        else srcs[0][:, k, lo:lo + n]
                    nc.tensor.matmul(ps[:, mh, :n], w1T[:, k, mh, :], src,
                                     start=(k == 0), stop=(k == nk - 1))
            for mh in range(2):
                nc.scalar.activation(ht[:, mh, :n], ps[:, mh, :n], AF.Lrelu,
                                     bias=bias[:, mh:mh + 1], alpha=LRELU_A)
            return ht

        for t in range(NT):
            xt = io.tile([128, 2, 512], F32R, tag="xt")
            nc.sync.dma_start(xt[:], xin.ap()[:, :, t * 512:(t + 1) * 512])
            yt = io.tile([128, 2, 512], F32R, tag="yt")
            nc.sync.dma_start(yt[:], yin.ap()[:, :, t * 512:(t + 1) * 512])

            # valid-row window within this tile
            e0, e1 = max(2, 4 * t), min(ER - 2, 4 * t + 4)
            lo, n = (e0 - 4 * t) * 128, (e1 - e0) * 128

            fhx = mlp1([xt, yt], w["fxw1T"], 4, w["bfx"], "fhx", hidF, F32R,
                       lo, n)
            fhy = mlp1([xt, yt], w["fyw1T"], 4, w["bfy"], "fhy", hidF, F32R,
                       lo, n)
            qhx = mlp1([xt], w["qw1T"], 2, w["bq"], "qhx", hidQ, BF16, lo, n)
            qhy = mlp1([yt], w["qw1T"], 2, w["bq"], "qhy", hidQ, BF16, lo, n)
            khx = mlp1([fhx], w["kxw1T"], 2, w["bkx"], "khx", hidQ, BF16,
                       0, n)
            khy = mlp1([fhy], w["kyw1T"], 2, w["bky"], "khy", hidQ, BF16,
                       0, n)
            vhx = mlp1([xt], w["vw1T"], 2, w["bv"], "vhx", hidV, F32R)
            vhy = mlp1([yt], w["vw1T"], 2, w["bv"], "vhy", hidV, F32R)

            # v = vhid @ vw2T (ext tokens), spill to padded DRAM grid
            for nm, vh, vsp in (("x", vhx, vsp_x), ("y", vhy, vsp_y)):
                ps = psA.tile([128, 2, 512], F32, tag="psA")
                for mh in range(2):
                    for k in range(2):
                        nc.tensor.matmul(ps[:, mh, :], w["vw2T"][:, k, mh, :],
                                         vh[:, k, :], start=(k == 0),
                                         stop=(k == 1))
                vt = io.tile([128, 2, 512], F32R, tag=f"vt{nm}")
                nc.vector.tensor_copy(vt[:], ps[:])
                nc.sync.dma_start(
                    vsp.ap().rearrange("p a (r c) -> p a r c", c=WP)
                    [:, :, 4 * t:4 * t + 4, 1:129],
                    vt[:])

            # token-major QK L2 + Gram per valid image row
            for e in range(e0, e1):
                off = (e - e0) * 128
                st = stk.tile([128, HEADS, 4, DH], BF16, tag="st")
                for src, (hh, w2T) in enumerate(
                        ((khy, "kw2T"), (qhx, "qw2T"),
                         (khx, "kw2T"), (qhy, "qw2T"))):
                    ps = psQ.tile([128, 256], F32, tag="psQ")
                    for k in range(2):
                        nc.tensor.matmul(ps[:], hh[:, k, off:off + 128],
                                         w[w2T][:, k, :], start=(k == 0),
                                         stop=(k == 1))
                    nc.vector.tensor_copy(
                        st[:, :, src, :],
                        ps.rearrange("p (h d) -> p h d", h=HEADS))
                for h in range(HEADS):
                    nc.tensor.matmul(
                        grams[h // 4][:, (h % 4) * 128:(h % 4) * 128 + 128],
                        st[:, h], st[:, h],
                        start=(vrow == 0), stop=(vrow == RB - 1),
                        skip_group_check=True)
                vrow += 1

            # interleaved conv1 chunks (only need earlier v rows)
            for g0, g1, after in C1CHUNKS:
                if after == t:
                    conv1_chunk(gx, vsp_x, g0, g1)
                    conv1_chunk(gy, vsp_y, g0, g1)

        # ================= Gram -> AllReduce =================
        gsb = sm.tile([128, 8, 128], F32, tag="gsb")
        for j in range(4):
            nc.vector.tensor_copy(gsb[:, 2 * j, :], grams[j // 2]
                                  [:, (j % 2) * 256:(j % 2) * 256 + 128])
            nc.vector.tensor_copy(
                gsb[:, 2 * j + 1, :],
                grams[j // 2][:, (j % 2) * 256 + 128:(j % 2) * 256 + 256])
        nc.sync.dma_start(cc_in.ap().rearrange("h d e -> d h e"), gsb[:])
        nc.gpsimd.collective_compute(
            "AllReduce", OP.add,
            ins=[cc_in.ap()], outs=[cc_out.ap()],
            replica_groups=[[0, 1, 2, 3], [4, 5, 6, 7]])

        # last conv1 chunk overlaps the collective
        for g0, g1, after in C1CHUNKS:
            if after is None:
                conv1_chunk(gx, vsp_x, g0, g1)
                conv1_chunk(gy, vsp_y, g0, g1)
        for gbuf in (gx, gy):
            nc.vector.tensor_scalar_mul(gbuf[:, :, 0, :], gbuf[:, :, 0, :],
                                        w["gm0"][:])
            nc.vector.tensor_scalar_mul(gbuf[:, :, ER - 3, :],
                                        gbuf[:, :, ER - 3, :], w["gm33"][:])

        # ================= softmax + BD + fused proj matrices ============
        m1ts = {}
        for d, (sl_d, sl_e, rexp, pwT) in {
            "x": (slice(0, 32), slice(32, 64), "rx_exp", "pxwT"),
            "y": (slice(64, 96), slice(96, 128), "ry_exp", "pywT"),
        }.items():
            s_t = sm.tile([128, 2, DH], F32, tag="s_t")
            nkq = sm.tile([128, 2, 2], F32, tag="nkq")
            for g in range(2):
                nc.sync.dma_start(s_t[:, g, :],
                                  cc_out.ap()[4 * g:4 * g + 4, sl_d, sl_e])
                for j, sl in enumerate((sl_d, sl_e)):
                    db = sm.tile([128, DH], F32, tag="db")
                    nc.sync.dma_start(db[:],
                                      cc_out.ap()[4 * g:4 * g + 4, sl, sl])
                    nc.vector.tensor_tensor(db[:], db[:], w["eye32r"][:],
                                            OP.mult)
                    nc.vector.tensor_reduce(nkq[:, g, j:j + 1], db[:],
                                            mybir.AxisListType.X, OP.add)
            inv = sm.tile([128, 2, 2], F32, tag="inv")
            nc.scalar.sqrt(inv[:], nkq[:])
            nc.vector.tensor_scalar_max(inv[:], inv[:], 1e-12)
            nc.vector.reciprocal(inv[:], inv[:])
            ks = sm.tile([128, 2], F32, tag="ks")
            nc.vector.tensor_tensor(ks[:], inv[:, :, 0], w[rexp][:], OP.mult)
            qs = sm.tile([128, 2, DH], F32, tag="qs")
            for g in range(2):
                eis = sm.tile([128, DH], F32, tag="eis")
                nc.vector.tensor_scalar_mul(eis[:], w["eye32r"][:],
                                            inv[:, g, 1:2])
                ei = sm.tile([128, DH], F32R, tag="ei")
                nc.vector.tensor_copy(ei[:], eis[:])
                pq = psQ.tile([128, DH], F32, tag="psQ")
                nc.tensor.matmul(pq[:], w["blk128"][:], ei[:],
                                 start=True, stop=True)
                nc.scalar.copy(qs[:, g, :], pq[:])
            lg = sm.tile([128, 2, DH], F32, tag="lg")
            for g in range(2):
                nc.vector.scalar_tensor_tensor(lg[:, g, :], s_t[:, g, :],
                                               ks[:, g:g + 1], qs[:, g, :],
                                               OP.mult, OP.mult)
            mx = sm.tile([128, 2], F32, tag="mx")
            nc.vector.tensor_reduce(mx[:], lg[:], mybir.AxisListType.X,
                                    OP.max)
            nc.vector.tensor_scalar_mul(mx[:], mx[:], -1.0)
            pe_ = sm.tile([128, 2, DH], F32, tag="pe_")
            ssum = sm.tile([128, 2], F32, tag="ssum")
            for g in range(2):
                nc.scalar.activation(pe_[:, g, :], lg[:, g, :], AF.Exp,
                                     bias=mx[:, g:g + 1],
                                     accum_out=ssum[:, g:g + 1])
            nc.vector.reciprocal(ssum[:], ssum[:])
            at = sm.tile([128, 2, DH], F32, tag="at")
            for g in range(2):
                nc.vector.tensor_scalar_mul(at[:, g, :], pe_[:, g, :],
                                            ssum[:, g:g + 1])
            bds = sm.tile([128, 2, 256], F32, tag="bds")
            nc.vector.memset(bds[:], 0.0)
            for g in range(2):
                for j in range(4):
                    h = 4 * g + j
                    nc.vector.tensor_copy(
                        bds[j * DH:(j + 1) * DH, g, h * DH:(h + 1) * DH],
                        at[j * DH:(j + 1) * DH, g, :])
            bd = sm.tile([128, 2, 256], F32R, tag="bd")
            nc.vector.tensor_copy(bd[:], bds[:])
            m1t = sm.tile([128, 2, 2, 128], F32R, tag=f"m1t_{d}")
            for me in range(2):
                ps = psQ.tile([128, 256], F32, tag="psQ")
                for g in range(2):
                    nc.tensor.matmul(ps[:],
                                     bd[:, g, me * 128:me * 128 + 128],
                                     w[pwT][:, g, :], start=(g == 0),
                                     stop=(g == 1))
                nc.scalar.copy(m1t[:, me, :, :],
                               ps.rearrange("p (a b) -> p a b", a=2))
            m1ts[d] = m1t

        # ========== final: (proj + conv2) fused in PSUM, store ==========
        for d, (vsp, gbuf, ob, o_dram) in {
            "x": (vsp_x, gx, "obx", out_x),
            "y": (vsp_y, gy, "oby", out_y),
        }.items():
            m1t = m1ts[d]
            for tt in range(8):
                vt = ot.tile([128, 2, 4 * WP], F32R, tag="vt_f")
                nc.sync.dma_start(
                    vt[:],
                    vsp.ap()[:, :, (4 * tt + 2) * WP:(4 * tt + 6) * WP])
                ps = psA.tile([128, 2, 512], F32, tag="psA")
                for mo in range(2):
                    for ke in range(2):
                        rhs = vt[:, ke, :].rearrange(
                            "p (r c) -> p r c", c=WP)[:, :, 1:129]
                        nc.tensor.matmul(ps[:, mo, :], m1t[:, ke, mo, :], rhs,
                                         start=(ke == 0), stop=False,
                                         skip_group_check=True)
                    for i in range(9):
                        dr, dc = TAPS[i]
                        src = gbuf[:, mo, 4 * tt + 1 + dr:4 * tt + 5 + dr,
                                   1 + dc:129 + dc]
                        nc.tensor.matmul(ps[:, mo, :], w["dw2"][:, mo, i, :],
                                         src, start=False, stop=(i == 8),
                                         skip_group_check=True)
                o_t = ot.tile([128, 2, 4, 128], F32, tag="o_t")
                for mo in range(2):
                    nc.scalar.activation(
                        o_t[:, mo, :, :],
                        ps[:, mo, :].rearrange("p (r c) -> p r c", c=128),
                        AF.Identity, bias=w[ob][:, mo:mo + 1])
                nc.sync.dma_start(
                    o_dram.ap()[:, :, tt * 512:(tt + 1) * 512],
                    o_t.rearrange("p a r c -> p a (r c)"))

    nc.finalize()
    return nc


# ======================= host side =======================

def _prep_core_input(full, b, h0):
    """(H, W, C) rows [h0-2, h0+34) -> channel-major [128, 2, EN] f32
    (zeros outside the image)."""
    arr = np.zeros((ER, W, C), np.float32)
    r0, r1 = h0 - 2, h0 + RB + 2
    cr0, cr1 = max(r0, 0), min(r1, H)
    arr[cr0 - r0:cr1 - r0] = full[b, cr0:cr1]
    cm = arr.transpose(2, 0, 1).reshape(2, 128, EN)
    return np.ascontiguousarray(cm.transpose(1, 0, 2))


def _cm(v):
    return np.ascontiguousarray(v.reshape(2, 128).T.astype(np.float32))


def _lhsT(wm, nk):
    t = wm.T.reshape(nk, 128, 2, 128)
    return np.ascontiguousarray(t.transpose(1, 0, 2, 3).astype(np.float32))


def _rhsT(wm, dt=np.float32):
    t = wm.T.reshape(2, 128, wm.shape[0])
    return np.ascontiguousarray(t.transpose(1, 0, 2).astype(dt))


def kernel(_trace=False, **inputs):
    inp = {k: np.asarray(v) for k, v in inputs.items()}
    bf = ml_dtypes.bfloat16

    w2c = inp["pe_w2"].reshape(256, 9).astype(np.float32)
    dw2 = np.zeros((128, 2, 9, 128), np.float32)
    for g in range(2):
        for t in range(9):
            dw2[np.arange(128), g, t, np.arange(128)] = \
                w2c[g * 128:(g + 1) * 128, t]

    shared = {
        "fxw1T": _lhsT(inp["fx_w1"], 4), "fyw1T": _lhsT(inp["fy_w1"], 4),
        "qw1T": _lhsT(inp["q_w1"], 2), "vw1T": _lhsT(inp["v_w1"], 2),
        "kxw1T": _lhsT(inp["k_w1"] @ inp["fx_w2"], 2),
        "kyw1T": _lhsT(inp["k_w1"] @ inp["fy_w2"], 2),
        "vw2T": _lhsT(inp["v_w2"], 2),
        "qw2T": _rhsT(inp["q_w2"], bf), "kw2T": _rhsT(inp["k_w2"], bf),
        "pxwT": _rhsT(inp["px_w"]), "pywT": _rhsT(inp["py_w"]),
        "dw2": dw2.astype(bf),
        "blk128": np.kron(np.eye(4), np.ones((32, 32))).astype(np.float32),
        "eye32r": np.tile(np.eye(32), (4, 1)).astype(np.float32),
        "bfx": _cm(inp["fx_b1"]), "bfy": _cm(inp["fy_b1"]),
        "bq": _cm(inp["q_b1"]), "bv": _cm(inp["v_b1"]),
        "bkx": _cm(inp["k_w1"] @ inp["fx_b2"] + inp["k_b1"]),
        "bky": _cm(inp["k_w1"] @ inp["fy_b2"] + inp["k_b1"]),
        "obx": _cm(inp["px_b"] + inp["pe_b2"]),
        "oby": _cm(inp["py_b"] + inp["pe_b2"]),
        "w1c": np.ascontiguousarray(
            inp["pe_w1"].reshape(256, 9).reshape(2, 128, 9)
            .transpose(1, 0, 2).astype(np.float32)),
        "b1c": _cm(inp["pe_b1"]),
        "rx_exp": np.ascontiguousarray(
            np.repeat(inp["rescale_x"].reshape(2, 4), 32, axis=1).T
            .astype(np.float32)),
        "ry_exp": np.ascontiguousarray(
            np.repeat(inp["rescale_y"].reshape(2, 4), 32, axis=1).T
            .astype(np.float32)),
    }

    in_maps = []
    for r in range(8):
        b, h0 = r // 4, (r % 4) * RB
        m = dict(shared)
        m["xin"] = _prep_core_input(inp["x_in"], b, h0)
        m["yin"] = _prep_core_input(inp["y_in"], b, h0)
        m["gm0"] = np.full((128, 1), 0.0 if h0 == 0 else 1.0, np.float32)
        m["gm33"] = np.full((128, 1), 0.0 if h0 + RB == H else 1.0,
                            np.float32)
        in_maps.append(m)

    if "nc" not in _CACHED:
        _CACHED["nc"] = _nc_build()
    res = run_bass_kernel_spmd(_CACHED["nc"], in_maps,
                               core_ids=list(range(8)), trace=_trace)
    _CACHED["last_result"] = res

    out_x = np.empty((B, H, W, C), np.float32)
    out_y = np.empty((B, H, W, C), np.float32)
    for r in range(8):
        b, h0 = r // 4, (r % 4) * RB
        for name, dst in (("out_x", out_x), ("out_y", out_y)):
            a = res.results[r][name].reshape(128, 2, RB, W)
            dst[b, h0:h0 + RB] = a.transpose(2, 3, 1, 0).reshape(RB, W, C)
    return out_x, out_y



# revision 5
# speedup vs baseline: 1.3166x; 1.3166x over previous
"""DMSA (dual-modal channel cross-attention) Trainium2 kernel — v3.

Sharding: 8 cores = 2 batches x 4 bands of 32 image rows. Each core
computes its band fully; the channel attention's per-head Gram matrices
(contraction over all n = h*w tokens, with l2-normalization folded in
via the Gram diagonal) are summed with one AllReduce per 4-core group.

Device layout: channel-major activations [128 partitions, 2 channel
halves, tokens]. Stage-1 runs on an unpadded 36x128 ext-row grid
(9 tiles x 512 tokens); v is spilled in fp16 to a width-padded 36x130
DRAM grid (pad columns kept zero inside the spill tile — no separate
pad-zero DMAs) so both 3x3 depthwise convs read taps as plain offset
views.

Engines: big matmuls f32r; q/k hidden + Gram in bf16; the v/conv path
(spill, conv1 on DVE, conv2 taps, final proj) in fp16 for 2x DVE rate
and half the DMA; conv1 interleaved into stage 1; conv2 folded into the
output projection's PSUM accumulation as 9 diagonal-matrix matmuls.
"""
import numpy as np
import ml_dtypes
from contextlib import ExitStack

import concourse.bass as bass
import concourse.tile as tile
import concourse.mybir as mybir
from concourse import bacc
from concourse.bass_utils import run_bass_kernel_spmd

F32 = mybir.dt.float32
F32R = mybir.dt.float32r
BF16 = mybir.dt.bfloat16
F16 = mybir.dt.float16
AF = mybir.ActivationFunctionType
OP = mybir.AluOpType

B, H, W, C = 2, 128, 128, 256
HEADS, DH = 8, 32
RB = 32             # image rows per core
ER = RB + 4         # ext rows
WP = W + 2          # padded width (conv grid)
GN = ER * WP        # padded tokens (v spill grid) = 4680
EN = ER * W         # unpadded ext tokens (stage-1 grid) = 4608
NV = RB * W         # valid tokens = 4096
NT = 9              # stage-1 tiles (4 ext rows each)
LRELU_A = 0.01
# conv1 chunk g-row ranges and the stage-1 tile after which each may run
C1CHUNKS = [(0, 6, 1), (6, 12, 3), (12, 18, 4), (18, 24, 6), (24, 30, 7),
            (30, 34, None)]  # None -> after the collective

_CACHED = {}


def _nc_build():
    nc = bacc.Bacc(num_devices=8)

    din = {}
    def inp(name, shape, dt=F32R):
        din[name] = nc.dram_tensor(name, list(shape), dt, kind="ExternalInput")
        return din[name]

    xin = inp("xin", [128, 2, EN])
    yin = inp("yin", [128, 2, EN])
    inp("fxw1T", [128, 4, 2, 128])
    inp("fyw1T", [128, 4, 2, 128])
    inp("qw1T", [128, 2, 2, 128])
    inp("kxw1T", [128, 2, 2, 128])
    inp("kyw1T", [128, 2, 2, 128])
    inp("vw1T", [128, 2, 2, 128])
    inp("vw2T", [128, 2, 2, 128])
    inp("qw2T", [128, 2, 256], BF16)
    inp("kw2T", [128, 2, 256], BF16)
    inp("pxwT", [128, 2, 256])
    inp("pywT", [128, 2, 256])
    inp("dw2", [128, 2, 9, 128], F16)        # conv2 taps as diagonal lhsT
    inp("blk128", [128, 128])                # kron(eye(4), ones(32,32))
    inp("eye32r", [128, 32], F32)            # tile(eye(32), (4,1))
    inp("eye8", [128, 8, 32], F32)           # eye32r repeated 8x (diag mask)
    for nm in ("bfx", "bfy", "bq", "bkx", "bky", "bv", "obx", "oby", "b1c"):
        inp(nm, [128, 2], F32)
    inp("rxy_exp", [128, 4], F32)            # rescale x(g0,g1), y(g0,g1)
    inp("w1c", [128, 2, 9], F32)             # conv1 taps (DVE)
    inp("gm0", [128, 1], F32)
    inp("gm33", [128, 1], F32)

    out_x = nc.dram_tensor("out_x", [128, 2, NV], F32, kind="ExternalOutput")
    out_y = nc.dram_tensor("out_y", [128, 2, NV], F32, kind="ExternalOutput")
    vsp_x = nc.dram_tensor("vsp_x", [128, 2, GN], F16, kind="Internal")
    vsp_y = nc.dram_tensor("vsp_y", [128, 2, GN], F16, kind="Internal")
    cc_in = nc.dram_tensor("cc_in", [HEADS, 128, 128], F32, kind="Internal")
    cc_out = nc.dram_tensor("cc_out", [HEADS, 128, 128], F32, kind="Internal")

    with tile.TileContext(nc) as tc, ExitStack() as ctx:
        wp = ctx.enter_context(tc.tile_pool(name="wp", bufs=1))
        io = ctx.enter_context(tc.tile_pool(name="io", bufs=2))
        hidF = ctx.enter_context(tc.tile_pool(name="hidF", bufs=2))
        hidQ = ctx.enter_context(tc.tile_pool(name="hidQ", bufs=2))
        hidV = ctx.enter_context(tc.tile_pool(name="hidV", bufs=2))
        stk = ctx.enter_context(tc.tile_pool(name="stk", bufs=2))
        sm = ctx.enter_context(tc.tile_pool(name="sm", bufs=1))
        gb = ctx.enter_context(tc.tile_pool(name="gb", bufs=1))
        cvp = ctx.enter_context(tc.tile_pool(name="cvp", bufs=2))
        ot = ctx.enter_context(tc.tile_pool(name="ot", bufs=2))
        psA = ctx.enter_context(tc.tile_pool(name="psA", bufs=2, space="PSUM"))
        psQ = ctx.enter_context(tc.tile_pool(name="psQ", bufs=2, space="PSUM"))
        psG = ctx.enter_context(tc.tile_pool(name="psG", bufs=1, space="PSUM"))

        w = {}
        for name, h in din.items():
            if name in ("xin", "yin"):
                continue
            t = wp.tile(list(h.shape), h.dtype, tag=f"w_{name}")
            nc.sync.dma_start(t[:], h.ap())
            w[name] = t

        gram0 = psG.tile([128, 512], F32, tag="gram0")
        gram1 = psG.tile([128, 512], F32, tag="gram1")
        grams = [gram0, gram1]

        gx = gb.tile([128, 2, ER - 2, WP], F16, tag="gx")
        gy = gb.tile([128, 2, ER - 2, WP], F16, tag="gy")
        nc.scalar.memzero(gx[:])
        nc.scalar.memzero(gy[:])
        TAPS = [(dr, dc) for dr in (-1, 0, 1) for dc in (-1, 0, 1)]

        # pre-zero the pad columns of both rotating spill tiles (their
        # interiors are overwritten each tile; borders stay zero)
        for nm in ("vtx", "vty"):
            for _ in range(2):
                vt = io.tile([128, 2, 4, WP], F16, tag=f"vt{nm[-1]}")
                nc.vector.memset(vt[:, :, :, 0:1], 0.0)
                nc.vector.memset(vt[:, :, :, WP - 1:WP], 0.0)

        def conv1_chunk(gbuf, vsp, g0, g1):
            """DVE 9-tap fp16 conv1 for g rows [g0, g1) + Gelu evict."""
            vr0, vr1 = g0, min(g1 + 2, ER)
            nr = g1 - g0
            vc = cvp.tile([128, 2, 8, WP], F16, tag="vc")
            nc.sync.dma_start(vc[:, :, :vr1 - vr0, :],
                              vsp.ap()[:, :, vr0 * WP:vr1 * WP])
            for g in range(2):
                acc = cvp.tile([128, 6, 128], F16, tag="cacc")
                for i, (dr, dc) in enumerate(TAPS):
                    src = vc[:, g, g0 + 1 + dr - vr0:g0 + 1 + dr - vr0 + nr,
                             1 + dc:129 + dc]
                    if i == 0:
                        nc.vector.tensor_scalar_mul(acc[:, :nr, :], src,
                                                    w["w1c"][:, g, 0:1])
                    else:
                        nc.vector.scalar_tensor_tensor(
                            acc[:, :nr, :], src, w["w1c"][:, g, i:i + 1],
                            acc[:, :nr, :], OP.mult, OP.add)
                nc.scalar.activation(gbuf[:, g, g0:g1, 1:129], acc[:, :nr, :],
                                     AF.Gelu, bias=w["b1c"][:, g:g + 1])

        # ================= stage 1 =================
        vrow = 0
        prev_st = None

        def emit_gram(st, row):
            for h in range(HEADS):
                nc.tensor.matmul(
                    grams[h // 4][:, (h % 4) * 128:(h % 4) * 128 + 128],
                    st[:, h], st[:, h],
                    start=(row == 0), stop=(row == RB - 1),
                    skip_group_check=True)

        def mlp1(srcs, w1T, nk, bias, tag, pool, dt, lo=0, n=512):
            """hidden = lrelu(srcs @ w1T + b); paired-bank PSUM."""
            ht = pool.tile([128, 2, 512], dt, tag=tag)
            ps = psA.tile([128, 2, 512], F32, tag="psA")
            for mh in range(2):
                for k in range(nk):
                    src = srcs[k // 2][:, k % 2, lo:lo + n] if len(srcs) > 1 \
                        else srcs[0][:, k, lo:lo + n]
                    nc.tensor.matmul(ps[:, mh, :n], w1T[:, k, mh, :], src,
                                     start=(k == 0), stop=(k == nk - 1))
            for mh in range(2):
                nc.scalar.activation(ht[:, mh, :n], ps[:, mh, :n], AF.Lrelu,
                                     bias=bias[:, mh:mh + 1], alpha=LRELU_A)
            return ht

        for t in range(NT):
            xt = io.tile([128, 2, 512], F32R, tag="xt")
            nc.sync.dma_start(xt[:], xin.ap()[:, :, t * 512:(t + 1) * 512])
            yt = io.tile([128, 2, 512], F32R, tag="yt")
            nc.sync.dma_start(yt[:], yin.ap()[:, :, t * 512:(t + 1) * 512])

            # valid-row window within this tile
            e0, e1 = max(2, 4 * t), min(ER - 2, 4 * t + 4)
            lo, n = (e0 - 4 * t) * 128, (e1 - e0) * 128

            fhx = mlp1([xt, yt], w["fxw1T"], 4, w["bfx"], "fhx", hidF, F32R,
                       lo, n)
            fhy = mlp1([xt, yt], w["fyw1T"], 4, w["bfy"], "fhy", hidF, F32R,
                       lo, n)
            qhx = mlp1([xt], w["qw1T"], 2, w["bq"], "qhx", hidQ, BF16, lo, n)
            qhy = mlp1([yt], w["qw1T"], 2, w["bq"], "qhy", hidQ, BF16, lo, n)
            khx = mlp1([fhx], w["kxw1T"], 2, w["bkx"], "khx", hidQ, BF16,
                       0, n)
            khy = mlp1([fhy], w["kyw1T"], 2, w["bky"], "khy", hidQ, BF16,
                       0, n)
            vhx = mlp1([xt], w["vw1T"], 2, w["bv"], "vhx", hidV, F32R)
            vhy = mlp1([yt], w["vw1T"], 2, w["bv"], "vhy", hidV, F32R)

            # v = vhid @ vw2T (ext tokens), fp16 spill to padded DRAM grid
            for nm, vh, vsp in (("x", vhx, vsp_x), ("y", vhy, vsp_y)):
                ps = psA.tile([128, 2, 512], F32, tag="psA")
                for mh in range(2):
                    for k in range(2):
                        nc.tensor.matmul(ps[:, mh, :], w["vw2T"][:, k, mh, :],
                                         vh[:, k, :], start=(k == 0),
                                         stop=(k == 1))
                vt = io.tile([128, 2, 4, WP], F16, tag=f"vt{nm}")
                for g in range(2):
                    nc.vector.tensor_copy(
                        vt[:, g, :, 1:129],
                        ps[:, g, :].rearrange("p (r c) -> p r c", c=128))
                nc.sync.dma_start(
                    vsp.ap().rearrange("p a (r c) -> p a r c", c=WP)
                    [:, :, 4 * t:4 * t + 4, :],
                    vt[:])

            # token-major QK L2 + Gram per valid image row; the Gram for a
            # row is emitted after the next row's w2 matmuls so PE has work
            # while the st evict runs
            for e in range(e0, e1):
                off = (e - e0) * 128
                st = stk.tile([128, HEADS, 4, DH], BF16, tag="st")
                for half in range(2):
                    ps = psQ.tile([128, 2, 256], F32, tag="psQ")
                    for s2 in range(2):
                        hh, w2T = ((khy, "kw2T"), (qhx, "qw2T"),
                                   (khx, "kw2T"), (qhy, "qw2T"))[half * 2 + s2]
                        for k in range(2):
                            nc.tensor.matmul(ps[:, s2, :],
                                             hh[:, k, off:off + 128],
                                             w[w2T][:, k, :], start=(k == 0),
                                             stop=(k == 1))
                    dst = st[:, :, half * 2:half * 2 + 2, :]
                    src = ps.rearrange("p s (h d) -> p h s d", h=HEADS)
                    if half == 0:
                        nc.scalar.copy(dst, src)
                    else:
                        nc.vector.tensor_copy(dst, src)
                if prev_st is not None:
                    emit_gram(*prev_st)
                prev_st = (st, vrow)
                vrow += 1

            # interleaved conv1 chunks (only need earlier v rows)
            for g0, g1, after in C1CHUNKS:
                if after == t:
                    conv1_chunk(gx, vsp_x, g0, g1)
                    conv1_chunk(gy, vsp_y, g0, g1)

        emit_gram(*prev_st)

        # ================= Gram -> AllReduce =================
        gsb = sm.tile([128, 8, 128], F32, tag="gsb")
        for j in range(4):
            nc.vector.tensor_copy(gsb[:, 2 * j, :], grams[j // 2]
                                  [:, (j % 2) * 256:(j % 2) * 256 + 128])
            nc.vector.tensor_copy(
                gsb[:, 2 * j + 1, :],
                grams[j // 2][:, (j % 2) * 256 + 128:(j % 2) * 256 + 256])
        nc.sync.dma_start(cc_in.ap().rearrange("h d e -> d h e"), gsb[:])
        nc.gpsimd.collective_compute(
            "AllReduce", OP.add,
            ins=[cc_in.ap()], outs=[cc_out.ap()],
            replica_groups=[[0, 1, 2, 3], [4, 5, 6, 7]])

        # last conv1 chunk overlaps the collective
        for g0, g1, after in C1CHUNKS:
            if after is None:
                conv1_chunk(gx, vsp_x, g0, g1)
                conv1_chunk(gy, vsp_y, g0, g1)
        for gbuf in (gx, gy):
            nc.vector.tensor_scalar_mul(gbuf[:, :, 0, :], gbuf[:, :, 0, :],
                                        w["gm0"][:])
            nc.vector.tensor_scalar_mul(gbuf[:, :, ER - 3, :],
                                        gbuf[:, :, ER - 3, :], w["gm33"][:])

        # ====== softmax + BD + fused proj matrices (x and y batched) ======
        # dg index: 0=(x,g0) 1=(x,g1) 2=(y,g0) 3=(y,g1)
        DG = [(0, slice(0, 32), slice(32, 64)),
              (1, slice(0, 32), slice(32, 64)),
              (2, slice(64, 96), slice(96, 128)),
              (3, slice(64, 96), slice(96, 128))]
        s_t = sm.tile([128, 4, DH], F32, tag="s_t")
        db = sm.tile([128, 4, 2, DH], F32, tag="db")
        for dg, sl_d, sl_e in DG:
            g = dg % 2
            nc.sync.dma_start(s_t[:, dg, :],
                              cc_out.ap()[4 * g:4 * g + 4, sl_d, sl_e])
            nc.sync.dma_start(db[:, dg, 0, :],
                              cc_out.ap()[4 * g:4 * g + 4, sl_d, sl_d])
            nc.sync.dma_start(db[:, dg, 1, :],
                              cc_out.ap()[4 * g:4 * g + 4, sl_e, sl_e])
        dbv = db.rearrange("p a b d -> p (a b) d")
        nc.vector.tensor_tensor(dbv[:], dbv[:], w["eye8"][:], OP.mult)
        nkq = sm.tile([128, 4, 2], F32, tag="nkq")
        nc.vector.tensor_reduce(nkq.rearrange("p a b -> p (a b)")[:],
                                dbv[:], mybir.AxisListType.X, OP.add)
        inv = sm.tile([128, 4, 2], F32, tag="inv")
        nc.scalar.sqrt(inv[:], nkq[:])
        nc.vector.tensor_scalar_max(inv[:], inv[:], 1e-12)
        nc.vector.reciprocal(inv[:], inv[:])
        ks = sm.tile([128, 4], F32, tag="ks")
        nc.vector.tensor_tensor(ks[:], inv[:, :, 0], w["rxy_exp"][:], OP.mult)
        # qs[p, dg, j] = 1/||q_(head group(p), j)|| broadcast via blk128
        ei = sm.tile([128, 4, DH], F32, tag="ei")
        for dg in range(4):
            nc.vector.tensor_scalar_mul(ei[:, dg, :], w["eye32r"][:],
                                        inv[:, dg, 1:2])
        eir = sm.tile([128, 4, DH], F32R, tag="eir")
        nc.vector.tensor_copy(eir[:], ei[:])
        pq = psQ.tile([128, 4, DH], F32, tag="psQ")
        nc.tensor.matmul(pq.rearrange("p a d -> p (a d)")[:], w["blk128"][:],
                         eir.rearrange("p a d -> p (a d)")[:],
                         start=True, stop=True)
        qks = sm.tile([128, 4, DH], F32, tag="qks")
        for dg in range(4):
            nc.scalar.activation(qks[:, dg, :], pq[:, dg, :], AF.Identity,
                                 scale=ks[:, dg:dg + 1])
        lg = sm.tile([128, 4, DH], F32, tag="lg")
        nc.vector.tensor_tensor(lg[:], s_t[:], qks[:], OP.mult)
        mx = sm.tile([128, 4], F32, tag="mx")
        nc.vector.tensor_reduce(mx[:], lg[:], mybir.AxisListType.X, OP.max)
        nc.vector.tensor_scalar_mul(mx[:], mx[:], -1.0)
        pe_ = sm.tile([128, 4, DH], F32, tag="pe_")
        ssum = sm.tile([128, 4], F32, tag="ssum")
        for dg in range(4):
            nc.scalar.activation(pe_[:, dg, :], lg[:, dg, :], AF.Exp,
                                 bias=mx[:, dg:dg + 1],
                                 accum_out=ssum[:, dg:dg + 1])
        nc.vector.reciprocal(ssum[:], ssum[:])
        at = sm.tile([128, 4, DH], F32, tag="at")
        for dg in range(4):
            nc.vector.tensor_scalar_mul(at[:, dg, :], pe_[:, dg, :],
                                        ssum[:, dg:dg + 1])
        m1ts = {}
        for d, (dgb, pwT) in {"x": (0, "pxwT"), "y": (2, "pywT")}.items():
            bds = sm.tile([128, 2, 256], F32, tag="bds")
            nc.vector.memset(bds[:], 0.0)
            for g in range(2):
                for j in range(4):
                    h = 4 * g + j
                    nc.vector.tensor_copy(
                        bds[j * DH:(j + 1) * DH, g, h * DH:(h + 1) * DH],
                        at[j * DH:(j + 1) * DH, dgb + g, :])
            bd = sm.tile([128, 2, 256], F32R, tag="bd")
            nc.vector.tensor_copy(bd[:], bds[:])
            m1t = sm.tile([128, 2, 2, 128], F16, tag=f"m1t_{d}")
            for me in range(2):
                ps = psQ.tile([128, 256], F32, tag="psQ")
                for g in range(2):
                    nc.tensor.matmul(ps[:],
                                     bd[:, g, me * 128:me * 128 + 128],
                                     w[pwT][:, g, :], start=(g == 0),
                                     stop=(g == 1))
                nc.scalar.copy(m1t[:, me, :, :],
                               ps.rearrange("p (a b) -> p a b", a=2))
            m1ts[d] = m1t

        # ========== final: (proj + conv2) fused in PSUM, store ==========
        for d, (vsp, gbuf, ob, o_dram) in {
            "x": (vsp_x, gx, "obx", out_x),
            "y": (vsp_y, gy, "oby", out_y),
        }.items():
            m1t = m1ts[d]
            for tt in range(8):
                vt = ot.tile([128, 2, 4 * WP], F16, tag="vt_f")
                nc.sync.dma_start(
                    vt[:],
                    vsp.ap()[:, :, (4 * tt + 2) * WP:(4 * tt + 6) * WP])
                ps = psA.tile([128, 2, 512], F32, tag="psA")
                for mo in range(2):
                    for ke in range(2):
                        rhs = vt[:, ke, :].rearrange(
                            "p (r c) -> p r c", c=WP)[:, :, 1:129]
                        nc.tensor.matmul(ps[:, mo, :], m1t[:, ke, mo, :], rhs,
                                         start=(ke == 0), stop=False,
                                         skip_group_check=True)
                    for i in range(9):
                        dr, dc = TAPS[i]
                        src = gbuf[:, mo, 4 * tt + 1 + dr:4 * tt + 5 + dr,
                                   1 + dc:129 + dc]
                        nc.tensor.matmul(ps[:, mo, :], w["dw2"][:, mo, i, :],
                                         src, start=False, stop=(i == 8),
                                         skip_group_check=True)
                o_t = ot.tile([128, 2, 4, 128], F32, tag="o_t")
                for mo in range(2):
                    nc.scalar.activation(
                        o_t[:, mo, :, :],
                        ps[:, mo, :].rearrange("p (r c) -> p r c", c=128),
                        AF.Identity, bias=w[ob][:, mo:mo + 1])
                nc.sync.dma_start(
                    o_dram.ap()[:, :, tt * 512:(tt + 1) * 512],
                    o_t.rearrange("p a r c -> p a (r c)"))

    nc.finalize()
    return nc


# ======================= host side =======================

def _prep_core_input(full, b, h0):
    """(H, W, C) rows [h0-2, h0+34) -> channel-major [128, 2, EN] f32
    (zeros outside the image)."""
    arr = np.zeros((ER, W, C), np.float32)
    r0, r1 = h0 - 2, h0 + RB + 2
    cr0, cr1 = max(r0, 0), min(r1, H)
    arr[cr0 - r0:cr1 - r0] = full[b, cr0:cr1]
    cm = arr.transpose(2, 0, 1).reshape(2, 128, EN)
    return np.ascontiguousarray(cm.transpose(1, 0, 2))


def _cm(v):
    return np.ascontiguousarray(v.reshape(2, 128).T.astype(np.float32))


def _lhsT(wm, nk):
    t = wm.T.reshape(nk, 128, 2, 128)
    return np.ascontiguousarray(t.transpose(1, 0, 2, 3).astype(np.float32))


def _rhsT(wm, dt=np.float32):
    t = wm.T.reshape(2, 128, wm.shape[0])
    return np.ascontiguousarray(t.transpose(1, 0, 2).astype(dt))


def kernel(_trace=False, **inputs):
    inp = {k: np.asarray(v) for k, v in inputs.items()}
    bf = ml_dtypes.bfloat16

    w2c = inp["pe_w2"].reshape(256, 9).astype(np.float32)
    dw2 = np.zeros((128, 2, 9, 128), np.float32)
    for g in range(2):
        for t in range(9):
            dw2[np.arange(128), g, t, np.arange(128)] = \
                w2c[g * 128:(g + 1) * 128, t]

    shared = {
        "fxw1T": _lhsT(inp["fx_w1"], 4), "fyw1T": _lhsT(inp["fy_w1"], 4),
        "qw1T": _lhsT(inp["q_w1"], 2), "vw1T": _lhsT(inp["v_w1"], 2),
        "kxw1T": _lhsT(inp["k_w1"] @ inp["fx_w2"], 2),
        "kyw1T": _lhsT(inp["k_w1"] @ inp["fy_w2"], 2),
        "vw2T": _lhsT(inp["v_w2"], 2),
        "qw2T": _rhsT(inp["q_w2"], bf), "kw2T": _rhsT(inp["k_w2"], bf),
        "pxwT": _rhsT(inp["px_w"]), "pywT": _rhsT(inp["py_w"]),
        "dw2": dw2.astype(np.float16),
        "blk128": np.kron(np.eye(4), np.ones((32, 32))).astype(np.float32),
        "eye32r": np.tile(np.eye(32), (4, 1)).astype(np.float32),
        "eye8": np.ascontiguousarray(
            np.broadcast_to(np.tile(np.eye(32), (4, 1))[:, None, :],
                            (128, 8, 32)).astype(np.float32)),
        "bfx": _cm(inp["fx_b1"]), "bfy": _cm(inp["fy_b1"]),
        "bq": _cm(inp["q_b1"]), "bv": _cm(inp["v_b1"]),
        "bkx": _cm(inp["k_w1"] @ inp["fx_b2"] + inp["k_b1"]),
        "bky": _cm(inp["k_w1"] @ inp["fy_b2"] + inp["k_b1"]),
        "obx": _cm(inp["px_b"] + inp["pe_b2"]),
        "oby": _cm(inp["py_b"] + inp["pe_b2"]),
        "w1c": np.ascontiguousarray(
            inp["pe_w1"].reshape(256, 9).reshape(2, 128, 9)
            .transpose(1, 0, 2).astype(np.float32)),
        "b1c": _cm(inp["pe_b1"]),
        "rxy_exp": np.ascontiguousarray(np.concatenate([
            np.repeat(inp["rescale_x"].reshape(2, 4), 32, axis=1).T,
            np.repeat(inp["rescale_y"].reshape(2, 4), 32, axis=1).T,
        ], axis=1).astype(np.float32)),
    }

    in_maps = []
    for r in range(8):
        b, h0 = r // 4, (r % 4) * RB
        m = dict(shared)
        m["xin"] = _prep_core_input(inp["x_in"], b, h0)
        m["yin"] = _prep_core_input(inp["y_in"], b, h0)
        m["gm0"] = np.full((128, 1), 0.0 if h0 == 0 else 1.0, np.float32)
        m["gm33"] = np.full((128, 1), 0.0 if h0 + RB == H else 1.0,
                            np.float32)
        in_maps.append(m)

    if "nc" not in _CACHED:
        _CACHED["nc"] = _nc_build()
    res = run_bass_kernel_spmd(_CACHED["nc"], in_maps,
                               core_ids=list(range(8)), trace=_trace)
    _CACHED["last_result"] = res

    out_x = np.empty((B, H, W, C), np.float32)
    out_y = np.empty((B, H, W, C), np.float32)
    for r in range(8):
        b, h0 = r // 4, (r % 4) * RB
        for name, dst in (("out_x", out_x), ("out_y", out_y)):
            a = res.results[r][name].reshape(128, 2, RB, W)
            dst[b, h0:h0 + RB] = a.transpose(2, 3, 1, 0).reshape(RB, W, C)
    return out_x, out_y
